# revision 14
# baseline (speedup 1.0000x reference)
"""Trainium2 Bass kernel for nn_CausalFieldLayer (v2).

Math (validated on host, see module docstring of the baseline):
  * W_in folds into the three 1024->16 projections -> one [1024,49] matrix
    (48 proj cols + a ones column for sum_d x, used by the layernorm mean).
  * The complex-octonion associator Jv is computed per 512-token chunk in
    channel-major layout: PE expands ps/pl/pa (and U, Y) into 256-row
    outer-product operands, DVE multiplies them, PE contracts by G2.
  * Everything downstream of Jv/Jc is linear and folds into pcat [66,1024];
    the 64-tap causal FFT conv is a Toeplitz matmul over token-major Jv.
  * Layernorm mean is folded into the final matmul as a -mu row; variance
    via ScalarE Square+accum; normalize on DVE.
  * Data-parallel over B=8: core i handles batch element i.

v2 layout/engine changes vs the baseline:
  * Projections run as ONE 8-matmul group into a contiguous pall [49,TC]
    (ps@0:16, pl@16:32, pa@32:48, sumx@48); the expansion stationaries are
    48-row masked matrices so no 32-aligned partition groups are needed.
  * The residual path is bf16: x is converted once on GPSIMD (idle engine),
    feeding both the PE transposes and the residual add; the final matmul
    accumulates into a bf16 PSUM tile. fp32 is only used for PSUM
    accumulation and the final normalized output.
  * Expansion pairs (h=0,1) share one [128,2,TC] bf16 PSUM tile -> one DVE
    mul per product (4/chunk instead of 8).
  * rep(ps) and tile(pa) are each used twice -> evacuated once to SBUF
    (DVE, int32-bitcast copy); the other expansion operand of each mul
    reads PSUM directly.
  * Per-chunk pooled tiles (TT, JvT) replace the persistent JJ tensor, so
    chunks pipeline without whole-tensor dependency serialization; the
    conv's one cross-chunk input is a tiny [128,16] "prev" tile.
"""

from contextlib import ExitStack

import numpy as np
import ml_dtypes

import concourse.bass as bass
import concourse.bacc as bacc
import concourse.mybir as mybir
import concourse.tile as tile
from concourse.bass_utils import run_bass_kernel_spmd

BF = ml_dtypes.bfloat16
F32 = np.float32

B, N, DM = 8, 4096, 1024
NCORES = 8
KSIZE = 64

EPS = 1e-5


# ----------------------------------------------------------------------------
# Host-side folding
# ----------------------------------------------------------------------------

def fold_params(inp):
    f64 = np.float64
    f = np.asarray(inp["oct_struct"], f64)  # [8,8,8] f[j,k,i]
    W_cat = np.concatenate(
        [np.asarray(inp[k], f64) for k in ("W_sigma", "W_lam", "W_alp")], axis=1
    )  # [1024,48]
    W_all = np.asarray(inp["W_in"], f64) @ W_cat
    b_all = np.asarray(inp["b_in"], f64) @ W_cat + np.concatenate(
        [np.asarray(inp[k], f64) for k in ("b_sigma", "b_lam", "b_alp")]
    )

    # cmul structure tensor G[i,j,k]: cmul(u,v)_i = sum_jk G[i,j,k] u_j v_k
    G = np.zeros((16, 16, 16), f64)
    ft = np.transpose(f, (2, 0, 1))  # ft[i,j,k] = f[j,k,i]
    G[:8, :8, :8] = ft
    G[:8, 8:, 8:] = -ft
    G[8:, :8, 8:] = ft
    G[8:, 8:, :8] = ft
    G2 = G.transpose(1, 2, 0).reshape(256, 16)  # [jk, i]

    JE = np.asarray(inp["J_expand"], f64)
    A = (JE - np.transpose(JE, (0, 2, 1))).reshape(16, 256)
    Gamma = np.einsum("ab,bcd->cd", np.asarray(inp["tetrad"], f64),
                      np.asarray(inp["gammas"], f64))
    sp = np.einsum("gdk,gd->k", np.asarray(inp["Pi_spinor"], f64), Gamma)
    PiS = np.asarray(inp["Pi_source"], f64).reshape(256, 16)
    PiT = np.asarray(inp["Pi_target"], f64).reshape(256, 16)
    C = (A @ PiS) @ PiT.T * np.tile(sp, 16)[None, :]

    kw = np.asarray(inp["kweights"], f64)
    alpha = kw[0]
    W_out = np.asarray(inp["W_out"], f64)
    P1 = alpha * (A @ W_out)
    P2 = (1.0 - alpha) * (C @ W_out)
    b_out = np.asarray(inp["b_out"], f64)

    # wcat [1024, 49]: ps@0:16, pl@16:32, pa@32:48, ones@48 (sum_d x)
    wcat = np.zeros((DM, 49), f64)
    wcat[:, 0:48] = W_all
    wcat[:, 48] = 1.0
    ball = np.zeros((49, 1), f64)
    ball[0:48, 0] = b_all

    # expansion stationaries reading contiguous pall[0:48]
    rep_ps48 = np.zeros((48, 256), f64)
    rep_pl48 = np.zeros((48, 256), f64)
    tile_pl48 = np.zeros((48, 256), f64)
    tile_pa48 = np.zeros((48, 256), f64)
    rep16 = np.zeros((16, 256), f64)
    tile16 = np.zeros((16, 256), f64)
    for j in range(16):
        for k in range(16):
            rep_ps48[j, j * 16 + k] = 1.0
            rep_pl48[16 + j, j * 16 + k] = 1.0
            tile_pl48[16 + k, j * 16 + k] = 1.0
            tile_pa48[32 + k, j * 16 + k] = 1.0
            rep16[j, j * 16 + k] = 1.0
            tile16[k, j * 16 + k] = 1.0

    # G2 chunks: [128, 4, 16] = [G2a, G2b, -G2a, -G2b]
    g2c = np.zeros((128, 4, 16), f64)
    g2c[:, 0] = G2[:128]
    g2c[:, 1] = G2[128:]
    g2c[:, 2] = -G2[:128]
    g2c[:, 3] = -G2[128:]

    # conv Toeplitz [192,128]: out[tl] = sum_sl afull[sl, tl] * Jv[t0-64+sl]
    afull = np.zeros((192, 128), f64)
    for sl in range(192):
        for tl in range(128):
            tap = tl + 64 - sl
            if 0 <= tap < KSIZE:
                afull[sl, tl] = kw[tap]
    a1p = np.zeros((128, 128), f64)
    a1p[64:128] = afull[0:64]  # stored at partition base 64
    a2 = afull[64:]

    # pcat [66, 1024]: 0:16 P1 (Jv), 32:48 P2 (Jc), 64 ones (-mu), 65 b_out
    pcat = np.zeros((66, DM), f64)
    pcat[0:16] = P1
    pcat[32:48] = P2
    pcat[64] = 1.0
    pcat[65] = b_out

    # svec [48,1]: row-sums of P1 at 0:16, of P2 at 32:48 (for sum_d out)
    svec = np.zeros((48, 1), f64)
    svec[0:16, 0] = P1.sum(axis=1)
    svec[32:48, 0] = P2.sum(axis=1)
    sumb = float(b_out.sum())

    # sel49 [49,1]: selects the sumx row (48) of pall
    sel49 = np.zeros((49, 1), f64)
    sel49[48, 0] = 1.0

    ln_g = np.asarray(inp["ln_g"], f64)
    ln_b = np.asarray(inp["ln_b"], f64)

    ballrow = ball.T  # [1, 49]

    return dict(
        ballrow=ballrow.astype(BF),
        ball_trivial=bool(np.all(ball == 0.0)),
        wcat=wcat.astype(BF),
        ball=ball.astype(F32),
        rep_ps48=rep_ps48.astype(BF),
        rep_pl48=rep_pl48.astype(BF),
        tile_pl48=tile_pl48.astype(BF),
        tile_pa48=tile_pa48.astype(BF),
        rep16=rep16.astype(BF),
        tile16=tile16.astype(BF),
        g2c=g2c.astype(BF),
        a1p=a1p.astype(BF),
        a2=a2.astype(BF),
        pcat=pcat.astype(BF),
        svec=svec.astype(BF),
        sel49=sel49.astype(BF),
        sumb=sumb,
        ident=np.eye(128).astype(BF),
        ln_g=ln_g.astype(F32),
        ln_b=ln_b.astype(F32),
        g_trivial=bool(np.all(ln_g == 1.0)),
        b_trivial=bool(np.all(ln_b == 0.0)),
    )


# ----------------------------------------------------------------------------
# Device kernel
# ----------------------------------------------------------------------------

def build_kernel(nc, T, sumb, g_trivial, b_trivial, reps=1,
                 ball_trivial=True):
    dt = mybir.dt
    P = 128
    TC = 512                 # token chunk
    TPC = TC // P            # token tiles per chunk (4)
    NCH = T // TC            # chunks

    x_d = nc.declare_dram_parameter("x", [T, DM], dt.float32, isOutput=False)
    y_d = nc.declare_dram_parameter("y", [T, DM], dt.float32, isOutput=True)
    wcat_d = nc.declare_dram_parameter("wcat", [DM, 49], dt.bfloat16, isOutput=False)
    ballrow_d = nc.declare_dram_parameter("ballrow", [1, 49], dt.bfloat16, isOutput=False)
    reps_d = nc.declare_dram_parameter("rep_ps48", [48, 256], dt.bfloat16, isOutput=False)
    repl_d = nc.declare_dram_parameter("rep_pl48", [48, 256], dt.bfloat16, isOutput=False)
    tilel_d = nc.declare_dram_parameter("tile_pl48", [48, 256], dt.bfloat16, isOutput=False)
    tilea_d = nc.declare_dram_parameter("tile_pa48", [48, 256], dt.bfloat16, isOutput=False)
    rep16_d = nc.declare_dram_parameter("rep16", [16, 256], dt.bfloat16, isOutput=False)
    tile16_d = nc.declare_dram_parameter("tile16", [16, 256], dt.bfloat16, isOutput=False)
    g2c_d = nc.declare_dram_parameter("g2c", [128, 4, 16], dt.bfloat16, isOutput=False)
    a1p_d = nc.declare_dram_parameter("a1p", [128, 128], dt.bfloat16, isOutput=False)
    a2_d = nc.declare_dram_parameter("a2", [128, 128], dt.bfloat16, isOutput=False)
    pcat_d = nc.declare_dram_parameter("pcat", [66, DM], dt.bfloat16, isOutput=False)
    svec_d = nc.declare_dram_parameter("svec", [48, 1], dt.bfloat16, isOutput=False)
    sel49_d = nc.declare_dram_parameter("sel49", [49, 1], dt.bfloat16, isOutput=False)
    ident_d = nc.declare_dram_parameter("ident", [128, 128], dt.bfloat16, isOutput=False)
    lng_d = nc.declare_dram_parameter("lng", [DM], dt.float32, isOutput=False)
    lnb_d = nc.declare_dram_parameter("lnb", [DM], dt.float32, isOutput=False)

    i32 = dt.int32

    with tile.TileContext(nc) as tc, ExitStack() as ctx:
        consts = ctx.enter_context(tc.tile_pool(name="consts", bufs=1))
        xin = ctx.enter_context(tc.tile_pool(name="xin", bufs=2))
        xbp = ctx.enter_context(tc.tile_pool(name="xbp", bufs=2))
        xtp = ctx.enter_context(tc.tile_pool(name="xtp", bufs=2))
        mid = ctx.enter_context(tc.tile_pool(name="mid", bufs=2))
        ttp = ctx.enter_context(tc.tile_pool(name="ttp", bufs=2))
        jvp = ctx.enter_context(tc.tile_pool(name="jvp", bufs=2))
        prv = ctx.enter_context(tc.tile_pool(name="prv", bufs=2))
        ycp = ctx.enter_context(tc.tile_pool(name="ycp", bufs=2))
        yop = ctx.enter_context(tc.tile_pool(name="yop", bufs=2))
        stat = ctx.enter_context(tc.tile_pool(name="stat", bufs=2))
        psB = ctx.enter_context(tc.tile_pool(name="psB", bufs=2, space="PSUM"))
        psE = ctx.enter_context(tc.tile_pool(name="psE", bufs=2, space="PSUM"))
        psS = ctx.enter_context(tc.tile_pool(name="psS", bufs=2, space="PSUM"))

        # ---- constants into SBUF ----
        wcat_sb = consts.tile([P, 8, 49], dt.bfloat16)
        nc.sync.dma_start(wcat_sb[:], wcat_d.rearrange("(a p) m -> p a m", p=P))
        ballrow_sb = ones1_sb = None
        if not ball_trivial:
            ballrow_sb = consts.tile([1, 49], dt.bfloat16)
            nc.sync.dma_start(ballrow_sb[:], ballrow_d[:])
            ones1_sb = consts.tile([1, TC], dt.bfloat16)
            nc.vector.memset(ones1_sb[:], 1.0)
        reps_sb = consts.tile([48, 256], dt.bfloat16)
        nc.sync.dma_start(reps_sb[:], reps_d[:])
        repl_sb = consts.tile([48, 256], dt.bfloat16)
        nc.sync.dma_start(repl_sb[:], repl_d[:])
        tilel_sb = consts.tile([48, 256], dt.bfloat16)
        nc.sync.dma_start(tilel_sb[:], tilel_d[:])
        tilea_sb = consts.tile([48, 256], dt.bfloat16)
        nc.sync.dma_start(tilea_sb[:], tilea_d[:])
        rep16_sb = consts.tile([16, 256], dt.bfloat16)
        nc.sync.dma_start(rep16_sb[:], rep16_d[:])
        tile16_sb = consts.tile([16, 256], dt.bfloat16)
        nc.sync.dma_start(tile16_sb[:], tile16_d[:])
        g2_sb = consts.tile([128, 4, 16], dt.bfloat16)
        nc.sync.dma_start(g2_sb[:], g2c_d[:])
        a1p_sb = consts.tile([128, 128], dt.bfloat16)
        nc.sync.dma_start(a1p_sb[:], a1p_d[:])
        a2_sb = consts.tile([128, 128], dt.bfloat16)
        nc.sync.dma_start(a2_sb[:], a2_d[:])
        pcat_sb = consts.tile([66, DM], dt.bfloat16)
        nc.sync.dma_start(pcat_sb[:], pcat_d[:])
        svec_sb = consts.tile([48, 1], dt.bfloat16)
        nc.sync.dma_start(svec_sb[:], svec_d[:])
        sel49_sb = consts.tile([49, 1], dt.bfloat16)
        nc.sync.dma_start(sel49_sb[:], sel49_d[:])
        ident_sb = consts.tile([128, 128], dt.bfloat16)
        nc.sync.dma_start(ident_sb[:], ident_d[:])
        eps_sb = consts.tile([P, 1], dt.float32)
        nc.vector.memset(eps_sb[:], EPS)

        gb_sb = None
        if not (g_trivial and b_trivial):
            gb_sb = consts.tile([P, 2, DM], dt.float32)
            nc.sync.dma_start(gb_sb[:, 0, :], lng_d[None, :].to_broadcast((P, DM)))
            nc.sync.dma_start(gb_sb[:, 1, :], lnb_d[None, :].to_broadcast((P, DM)))

        rep_cm = tc.For_i(0, reps, 1) if reps > 1 else None
        if rep_cm is not None:
            rep_cm.__enter__()

        prev_t = None  # [128,16] bf16: last 128-token Jv tile of prior chunk

        for c in range(NCH):
            t0 = c * TC
            csl = slice(t0, t0 + TC)

            # ---- load x chunk, convert to bf16 on GPSIMD ----
            x32 = xin.tile([P, TPC, DM], dt.float32)
            nc.sync.dma_start(
                x32[:], x_d[csl, :].rearrange("(j p) d -> p j d", p=P)
            )
            xb = xbp.tile([P, TPC, DM], dt.bfloat16)
            nc.gpsimd.tensor_copy(xb[:], x32[:])

            # ---- transpose x (PE) -> xT [128, 8, TC] bf16 ----
            xT = xtp.tile([P, 8, TC], dt.bfloat16)
            for j in range(TPC):
                pxT = psB.tile([P, 8, 128], dt.bfloat16, tag="psB")
                for a in range(8):
                    nc.tensor.transpose(
                        pxT[:, a, :], xb[:, j, bass.ts(a, 128)], ident_sb[:]
                    )
                nc.scalar.copy(xT[:, :, bass.ts(j, 128)], pxT[:])

            # ---- projections: ONE group -> pall [49, TC] ----
            pps = psS.tile([49, TC], dt.float32, tag="psS")
            nsteps = 8 if ball_trivial else 9
            for a in range(8):
                nc.tensor.matmul(
                    pps[:], wcat_sb[:, a, :], xT[:, a, :],
                    start=(a == 0), stop=(a == nsteps - 1),
                )
            if not ball_trivial:
                # bias via rank-1 matmul: ones[1,TC] x ballrow[1,49]
                nc.tensor.matmul(
                    pps[:], ballrow_sb[:], ones1_sb[:],
                    start=False, stop=True,
                )
            pall = mid.tile([49, TC], dt.bfloat16, tag="pall")
            nc.scalar.copy(pall[:], pps[:])
            p48 = pall[0:48, :]

            # ---- expansions (PE) ----
            def expand(stat_sb, src_ap, nrows):
                ps_t = psE.tile([P, 2, TC], dt.float32, tag="psE")
                for h in range(2):
                    nc.tensor.matmul(
                        ps_t[:, h, :], stat_sb[0:nrows, bass.ts(h, 128)], src_ap
                    )
                return ps_t

            p_rep_ps = expand(reps_sb, p48, 48)
            p_tile_pl = expand(tilel_sb, p48, 48)
            p_rep_pl = expand(repl_sb, p48, 48)
            p_tile_pa = expand(tilea_sb, p48, 48)

            # reused operands -> SBUF bf16 (ACT evacs)
            s_rep_ps = mid.tile([P, 2, TC], dt.bfloat16, tag="srp")
            nc.scalar.copy(s_rep_ps[:], p_rep_ps[:])
            s_tile_pa = mid.tile([P, 2, TC], dt.bfloat16, tag="stp")
            nc.scalar.copy(s_tile_pa[:], p_tile_pa[:])

            # ---- first-level products + contractions ----
            w1 = mid.tile([P, 2, TC], dt.bfloat16, tag="w1")
            nc.vector.tensor_mul(w1[:], s_rep_ps[:], p_tile_pl[:])
            w2 = mid.tile([P, 2, TC], dt.bfloat16, tag="w2")
            nc.vector.tensor_mul(w2[:], p_rep_pl[:], s_tile_pa[:])

            pU = psS.tile([16, TC], dt.float32, tag="psS")
            nc.tensor.matmul(pU[:], g2_sb[:, 0, :], w1[:, 0, :],
                             start=True, stop=False)
            nc.tensor.matmul(pU[:], g2_sb[:, 1, :], w1[:, 1, :],
                             start=False, stop=True)
            pY = psS.tile([16, TC], dt.float32, tag="psS")
            nc.tensor.matmul(pY[:], g2_sb[:, 0, :], w2[:, 0, :],
                             start=True, stop=False)
            nc.tensor.matmul(pY[:], g2_sb[:, 1, :], w2[:, 1, :],
                             start=False, stop=True)
            UY_sb = mid.tile([16, 2, TC], dt.bfloat16, tag="UY")
            nc.scalar.copy(UY_sb[:, 0, :], pU[:])
            nc.scalar.copy(UY_sb[:, 1, :], pY[:])

            # ---- second level: w3 = rep(U)*tile(pa), w4 = rep(ps)*tile(Y) ----
            p_rep_U = expand(rep16_sb, UY_sb[:, 0, :], 16)
            p_tile_Y = expand(tile16_sb, UY_sb[:, 1, :], 16)
            w3 = mid.tile([P, 2, TC], dt.bfloat16, tag="w3")
            nc.vector.tensor_mul(w3[:], p_rep_U[:], s_tile_pa[:])
            w4 = mid.tile([P, 2, TC], dt.bfloat16, tag="w4")
            nc.vector.tensor_mul(w4[:], s_rep_ps[:], p_tile_Y[:])

            pJv = psS.tile([16, TC], dt.float32, tag="psS")
            for i, (gi, w_t, h) in enumerate(
                [(0, w3, 0), (1, w3, 1), (2, w4, 0), (3, w4, 1)]
            ):
                nc.tensor.matmul(
                    pJv[:], g2_sb[:, gi, :], w_t[:, h, :],
                    start=(i == 0), stop=(i == 3),
                )

            # ---- TT tile: Jv@0:16, Jc@32:48, -mu@64, ones@65 ----
            TT = ttp.tile([66, TC], dt.bfloat16, tag="TT")
            nc.gpsimd.memset(TT[:], 0.0)
            nc.gpsimd.memset(TT[64:66, :], 1.0)
            nc.scalar.copy(TT[0:16, :], pJv[:])

            # ---- token-major Jv (for conv stationary) ----
            pjvT = psS.tile([P, TPC, 16], dt.bfloat16, tag="psS")
            for j in range(TPC):
                nc.tensor.transpose(
                    pjvT[:, j, :],
                    TT[0:16, bass.ts(j, 128)],
                    ident_sb[0:16, 0:16],
                )
            JvT = jvp.tile([P, TPC, 16], dt.bfloat16, tag="JvT")
            nc.scalar.copy(JvT[:], pjvT[:])

            # ---- causal conv (Toeplitz matmuls) -> Jc at rows 32:48 ----
            pJc = psS.tile([48, TC], dt.float32, tag="psS")
            for j in range(TPC):
                osl = pJc[32:48, bass.ts(j, 128)]
                if j > 0:
                    prev_stat = JvT[64:128, j - 1, :]
                elif prev_t is not None:
                    prev_stat = prev_t[64:128, :]
                else:
                    prev_stat = None
                if prev_stat is not None:
                    nc.tensor.matmul(
                        osl, prev_stat, a1p_sb[64:128, :],
                        start=True, stop=False, tile_position=(64, 32),
                    )
                    nc.tensor.matmul(
                        osl, JvT[:, j, :], a2_sb[:],
                        start=False, stop=True, tile_position=(0, 32),
                    )
                else:
                    nc.tensor.matmul(
                        osl, JvT[:, j, :], a2_sb[:],
                        start=True, stop=True, tile_position=(0, 32),
                    )
            nc.scalar.copy(TT[32:48, :], pJc[32:48, :])

            # save the chunk's last token tile for the next chunk's conv
            prev_t = prv.tile([P, 16], dt.bfloat16, tag="prev")
            nc.scalar.copy(prev_t[:], JvT[:, TPC - 1, :])

            # ---- mean: -mu = -(sum_d x + sum_d out + sum(b_out))/DM ----
            pmu = psS.tile([65, TC], dt.float32, tag="psS")
            nc.tensor.matmul(
                pmu[64:65, :], svec_sb[:], TT[0:48, :],
                start=True, stop=False, tile_position=(0, 64),
            )
            nc.tensor.matmul(
                pmu[64:65, :], sel49_sb[:], pall[:],
                start=False, stop=True, tile_position=(0, 64),
            )
            nc.scalar.activation(
                TT[64:65, :], pmu[64:65, :],
                mybir.ActivationFunctionType.Copy,
                bias=-sumb / DM, scale=-1.0 / DM,
            )

            # ---- final matmul (bf16 PSUM) + residual + layernorm ----
            ycb = ycp.tile([P, TPC, DM], dt.bfloat16)
            yout = yop.tile([P, TPC, DM], dt.float32)
            ssq = stat.tile([P, TPC], dt.float32, tag="ssq")
            sqs = stat.tile([P, DM], dt.bfloat16, tag="sqs")
            for j in range(TPC):
                for nh in range(2):
                    pout = psB.tile([P, 512], dt.float32, tag="psB")
                    nc.tensor.matmul(
                        pout[:],
                        TT[:, bass.ts(j, 128)],
                        pcat_sb[:, bass.ts(nh, 512)],
                    )
                    nc.vector.tensor_add(
                        ycb[:, j, bass.ts(nh, 512)],
                        xb[:, j, bass.ts(nh, 512)],
                        pout[:],
                    )
                nc.scalar.activation(
                    sqs[:], ycb[:, j, :], mybir.ActivationFunctionType.Square,
                    accum_out=ssq[:, j:j + 1],
                )

            sig4 = stat.tile([P, TPC], dt.float32, tag="sig4")
            nc.scalar.activation(
                sig4[:], ssq[:], mybir.ActivationFunctionType.Sqrt,
                bias=eps_sb[:], scale=1.0 / DM,
            )
            rsig4 = stat.tile([P, TPC], dt.float32, tag="rsig4")
            nc.vector.reciprocal(rsig4[:], sig4[:])

            for j in range(TPC):
                nc.gpsimd.tensor_scalar_mul(
                    yout[:, j, :], ycb[:, j, :], rsig4[:, j:j + 1]
                )
                if gb_sb is not None:
                    nc.vector.tensor_mul(yout[:, j, :], yout[:, j, :],
                                         gb_sb[:, 0, :])
                    nc.vector.tensor_add(yout[:, j, :], yout[:, j, :],
                                         gb_sb[:, 1, :])

            nc.sync.dma_start(
                y_d[csl, :].rearrange("(j p) d -> p j d", p=P), yout[:]
            )

        if rep_cm is not None:
            rep_cm.__exit__(None, None, None)

    return nc


# ----------------------------------------------------------------------------
# Entry point
# ----------------------------------------------------------------------------

def _const_map(fp):
    return {
        "wcat": fp["wcat"], "ballrow": fp["ballrow"],
        "rep_ps48": fp["rep_ps48"], "rep_pl48": fp["rep_pl48"],
        "tile_pl48": fp["tile_pl48"], "tile_pa48": fp["tile_pa48"],
        "rep16": fp["rep16"], "tile16": fp["tile16"],
        "g2c": fp["g2c"], "a1p": fp["a1p"], "a2": fp["a2"],
        "pcat": fp["pcat"], "svec": fp["svec"], "sel49": fp["sel49"],
        "ident": fp["ident"], "lng": fp["ln_g"], "lnb": fp["ln_b"],
    }


def _run(inputs, trace=False):
    x = inputs["x"]
    assert x.shape == (B, N, DM), x.shape
    fp = fold_params(inputs)

    nc = bacc.Bacc("TRN2", target_bir_lowering=False)
    build_kernel(nc, N, fp["sumb"], fp["g_trivial"], fp["b_trivial"],
                 ball_trivial=fp["ball_trivial"])
    nc.finalize()

    cm = _const_map(fp)
    in_maps = [
        {"x": np.ascontiguousarray(x[i], dtype=F32), **cm} for i in range(NCORES)
    ]
    return run_bass_kernel_spmd(nc, in_maps, list(range(NCORES)), trace=trace)


def kernel(**inputs):
    inputs = {k: np.asarray(v) for k, v in inputs.items()}
    res = _run(inputs)
    y = np.stack([res.results[i]["y"] for i in range(NCORES)], axis=0)
    return y.astype(np.float32)


def timed_run(inputs):
    """NTFF profiling is unavailable under axon in this container; timing is
    done by test.py via repeated execution of an in-kernel repeat loop."""
    return None


if __name__ == "__main__":
    import reference

    inp = reference.setup_inputs()
    out = kernel(**{k: np.asarray(v) for k, v in inp.items()})
    print("kernel output", out.shape, out.dtype)


# revision 27
# speedup vs baseline: 2.0345x; 2.0345x over previous
"""Trainium2 Bass kernel for nn_CausalFieldLayer (v2).

Math (validated on host, see module docstring of the baseline):
  * W_in folds into the three 1024->16 projections -> one [1024,49] matrix
    (48 proj cols + a ones column for sum_d x, used by the layernorm mean).
  * The complex-octonion associator Jv is computed per 512-token chunk in
    channel-major layout: PE expands ps/pl/pa (and U, Y) into 256-row
    outer-product operands, DVE multiplies them, PE contracts by G2.
  * Everything downstream of Jv/Jc is linear and folds into pcat [66,1024];
    the 64-tap causal FFT conv is a Toeplitz matmul over token-major Jv.
  * Layernorm mean is folded into the final matmul as a -mu row; variance
    via ScalarE Square+accum; normalize on DVE.
  * Data-parallel over B=8: core i handles batch element i.

v2 layout/engine changes vs the baseline:
  * Projections run as ONE 8-matmul group into a contiguous pall [49,TC]
    (ps@0:16, pl@16:32, pa@32:48, sumx@48); the expansion stationaries are
    48-row masked matrices so no 32-aligned partition groups are needed.
  * The residual path is bf16: x is converted once on GPSIMD (idle engine),
    feeding both the PE transposes and the residual add; the final matmul
    accumulates into a bf16 PSUM tile. fp32 is only used for PSUM
    accumulation and the final normalized output.
  * Expansion pairs (h=0,1) share one [128,2,TC] bf16 PSUM tile -> one DVE
    mul per product (4/chunk instead of 8).
  * rep(ps) and tile(pa) are each used twice -> evacuated once to SBUF
    (DVE, int32-bitcast copy); the other expansion operand of each mul
    reads PSUM directly.
  * Per-chunk pooled tiles (TT, JvT) replace the persistent JJ tensor, so
    chunks pipeline without whole-tensor dependency serialization; the
    conv's one cross-chunk input is a tiny [128,16] "prev" tile.
"""

from contextlib import ExitStack

import numpy as np
import ml_dtypes

import concourse.bass as bass
import concourse.bacc as bacc
import concourse.mybir as mybir
import concourse.tile as tile
from concourse.bass_utils import run_bass_kernel_spmd

BF = ml_dtypes.bfloat16
F32 = np.float32

B, N, DM = 8, 4096, 1024
NCORES = 8
KSIZE = 64

EPS = 1e-5


# ----------------------------------------------------------------------------
# Host-side folding
# ----------------------------------------------------------------------------

def fold_params(inp):
    f64 = np.float64
    f = np.asarray(inp["oct_struct"], f64)  # [8,8,8] f[j,k,i]
    W_cat = np.concatenate(
        [np.asarray(inp[k], f64) for k in ("W_sigma", "W_lam", "W_alp")], axis=1
    )  # [1024,48]
    W_all = np.asarray(inp["W_in"], f64) @ W_cat
    b_all = np.asarray(inp["b_in"], f64) @ W_cat + np.concatenate(
        [np.asarray(inp[k], f64) for k in ("b_sigma", "b_lam", "b_alp")]
    )

    # cmul structure tensor G[i,j,k]: cmul(u,v)_i = sum_jk G[i,j,k] u_j v_k
    G = np.zeros((16, 16, 16), f64)
    ft = np.transpose(f, (2, 0, 1))  # ft[i,j,k] = f[j,k,i]
    G[:8, :8, :8] = ft
    G[:8, 8:, 8:] = -ft
    G[8:, :8, 8:] = ft
    G[8:, 8:, :8] = ft
    G2 = G.transpose(1, 2, 0).reshape(256, 16)  # [jk, i]

    JE = np.asarray(inp["J_expand"], f64)
    A = (JE - np.transpose(JE, (0, 2, 1))).reshape(16, 256)
    Gamma = np.einsum("ab,bcd->cd", np.asarray(inp["tetrad"], f64),
                      np.asarray(inp["gammas"], f64))
    sp = np.einsum("gdk,gd->k", np.asarray(inp["Pi_spinor"], f64), Gamma)
    PiS = np.asarray(inp["Pi_source"], f64).reshape(256, 16)
    PiT = np.asarray(inp["Pi_target"], f64).reshape(256, 16)
    C = (A @ PiS) @ PiT.T * np.tile(sp, 16)[None, :]

    kw = np.asarray(inp["kweights"], f64)
    alpha = kw[0]
    W_out = np.asarray(inp["W_out"], f64)
    P1 = alpha * (A @ W_out)
    P2 = (1.0 - alpha) * (C @ W_out)
    b_out = np.asarray(inp["b_out"], f64)

    # wcat [1024, 49]: ps@0:16, pl@16:32, pa@32:48, ones@48 (sum_d x)
    wcat = np.zeros((DM, 49), f64)
    wcat[:, 0:48] = W_all
    wcat[:, 48] = 1.0
    ball = np.zeros((49, 1), f64)
    ball[0:48, 0] = b_all

    # expansion stationaries reading contiguous pall[0:48]
    rep_ps48 = np.zeros((48, 256), f64)
    rep_pl48 = np.zeros((48, 256), f64)
    tile_pl48 = np.zeros((48, 256), f64)
    tile_pa48 = np.zeros((48, 256), f64)
    rep16 = np.zeros((16, 256), f64)
    tile16 = np.zeros((16, 256), f64)
    for j in range(16):
        for k in range(16):
            rep_ps48[j, j * 16 + k] = 1.0
            rep_pl48[16 + j, j * 16 + k] = 1.0
            tile_pl48[16 + k, j * 16 + k] = 1.0
            tile_pa48[32 + k, j * 16 + k] = 1.0
            rep16[j, j * 16 + k] = 1.0
            tile16[k, j * 16 + k] = 1.0

    # G2 chunks: [128, 4, 16] = [G2a, G2b, -G2a, -G2b]
    g2c = np.zeros((128, 4, 16), f64)
    g2c[:, 0] = G2[:128]
    g2c[:, 1] = G2[128:]
    g2c[:, 2] = -G2[:128]
    g2c[:, 3] = -G2[128:]

    # conv Toeplitz [192,128]: out[tl] = sum_sl afull[sl, tl] * Jv[t0-64+sl]
    afull = np.zeros((192, 128), f64)
    for sl in range(192):
        for tl in range(128):
            tap = tl + 64 - sl
            if 0 <= tap < KSIZE:
                afull[sl, tl] = kw[tap]
    a1p = np.zeros((128, 128), f64)
    a1p[64:128] = afull[0:64]  # stored at partition base 64
    a2 = afull[64:]

    # pcat [66, 1024]: 0:16 P1 (Jv), 32:48 P2 (Jc), 64 ones (-mu), 65 b_out
    pcat = np.zeros((66, DM), f64)
    pcat[0:16] = P1
    pcat[32:48] = P2
    pcat[64] = 1.0
    pcat[65] = b_out

    # svec [48,1]: row-sums of P1 at 0:16, of P2 at 32:48 (for sum_d out)
    svec = np.zeros((48, 1), f64)
    svec[0:16, 0] = P1.sum(axis=1)
    svec[32:48, 0] = P2.sum(axis=1)
    sumb = float(b_out.sum())

    # sel49 [49,1]: selects the sumx row (48) of pall
    sel49 = np.zeros((49, 1), f64)
    sel49[48, 0] = 1.0

    ln_g = np.asarray(inp["ln_g"], f64)
    ln_b = np.asarray(inp["ln_b"], f64)

    ballrow = ball.T  # [1, 49]

    return dict(
        ballrow=ballrow.astype(BF),
        ball_trivial=bool(np.all(ball == 0.0)),
        wcat=wcat.astype(BF),
        ball=ball.astype(F32),
        rep_ps48=rep_ps48.astype(BF),
        rep_pl48=rep_pl48.astype(BF),
        tile_pl48=tile_pl48.astype(BF),
        tile_pa48=tile_pa48.astype(BF),
        rep16=rep16.astype(BF),
        tile16=tile16.astype(BF),
        g2c=g2c.astype(BF),
        a1p=a1p.astype(BF),
        a2=a2.astype(BF),
        pcat=pcat.astype(BF),
        svec=svec.astype(BF),
        sel49=sel49.astype(BF),
        sumb=sumb,
        ident=np.eye(128).astype(BF),
        ln_g=ln_g.astype(F32),
        ln_b=ln_b.astype(F32),
        g_trivial=bool(np.all(ln_g == 1.0)),
        b_trivial=bool(np.all(ln_b == 0.0)),
    )


# ----------------------------------------------------------------------------
# Device kernel
# ----------------------------------------------------------------------------

def build_kernel(nc, T, sumb, g_trivial, b_trivial, reps=1,
                 ball_trivial=True, sb_bufs=2, cfg=None):
    cfg = cfg or {}
    dt = mybir.dt
    P = 128
    TC = 512                 # token chunk
    TPC = TC // P            # token tiles per chunk (4)
    NCH = T // TC            # chunks

    x_d = nc.declare_dram_parameter("x", [T, DM], dt.float32, isOutput=False)
    y_d = nc.declare_dram_parameter("y", [T, DM], dt.float32, isOutput=True)
    wcat_d = nc.declare_dram_parameter("wcat", [DM, 49], dt.bfloat16, isOutput=False)
    ballrow_d = nc.declare_dram_parameter("ballrow", [1, 49], dt.bfloat16, isOutput=False)
    reps_d = nc.declare_dram_parameter("rep_ps48", [48, 256], dt.bfloat16, isOutput=False)
    repl_d = nc.declare_dram_parameter("rep_pl48", [48, 256], dt.bfloat16, isOutput=False)
    tilel_d = nc.declare_dram_parameter("tile_pl48", [48, 256], dt.bfloat16, isOutput=False)
    tilea_d = nc.declare_dram_parameter("tile_pa48", [48, 256], dt.bfloat16, isOutput=False)
    rep16_d = nc.declare_dram_parameter("rep16", [16, 256], dt.bfloat16, isOutput=False)
    tile16_d = nc.declare_dram_parameter("tile16", [16, 256], dt.bfloat16, isOutput=False)
    g2c_d = nc.declare_dram_parameter("g2c", [128, 4, 16], dt.bfloat16, isOutput=False)
    a1p_d = nc.declare_dram_parameter("a1p", [128, 128], dt.bfloat16, isOutput=False)
    a2_d = nc.declare_dram_parameter("a2", [128, 128], dt.bfloat16, isOutput=False)
    pcat_d = nc.declare_dram_parameter("pcat", [66, DM], dt.bfloat16, isOutput=False)
    svec_d = nc.declare_dram_parameter("svec", [48, 1], dt.bfloat16, isOutput=False)
    sel49_d = nc.declare_dram_parameter("sel49", [49, 1], dt.bfloat16, isOutput=False)
    ident_d = nc.declare_dram_parameter("ident", [128, 128], dt.bfloat16, isOutput=False)
    lng_d = nc.declare_dram_parameter("lng", [DM], dt.float32, isOutput=False)
    lnb_d = nc.declare_dram_parameter("lnb", [DM], dt.float32, isOutput=False)

    i32 = dt.int32

    with tile.TileContext(nc) as tc, ExitStack() as ctx:
        consts = ctx.enter_context(tc.tile_pool(name="consts", bufs=1))
        xin = ctx.enter_context(tc.tile_pool(name="xin", bufs=cfg.get("xin", 2)))
        xbp = ctx.enter_context(tc.tile_pool(name="xbp", bufs=cfg.get("xbp", 3)))
        xtp = ctx.enter_context(tc.tile_pool(name="xtp", bufs=cfg.get("xtp", 2)))
        mid = ctx.enter_context(tc.tile_pool(name="mid", bufs=cfg.get("mid", 3)))
        ttp = ctx.enter_context(tc.tile_pool(name="ttp", bufs=cfg.get("ttp", 3)))
        jvp = ctx.enter_context(tc.tile_pool(name="jvp", bufs=cfg.get("jvp", 3)))
        prv = ctx.enter_context(tc.tile_pool(name="prv", bufs=cfg.get("prv", 2)))
        ycp = ctx.enter_context(tc.tile_pool(name="ycp", bufs=cfg.get("ycp", 4)))
        yop = ctx.enter_context(tc.tile_pool(name="yop", bufs=cfg.get("yop", 4)))
        stat = ctx.enter_context(tc.tile_pool(name="stat", bufs=cfg.get("stat", 4)))
        psX = ctx.enter_context(tc.tile_pool(name="psX", bufs=cfg.get("psX", 1), space="PSUM"))
        psE = ctx.enter_context(tc.tile_pool(name="psE", bufs=cfg.get("psE", 2), space="PSUM"))
        psP = ctx.enter_context(tc.tile_pool(name="psP", bufs=cfg.get("psP", 1), space="PSUM"))
        psS = ctx.enter_context(tc.tile_pool(name="psS", bufs=cfg.get("psS", 1), space="PSUM"))
        psO = ctx.enter_context(tc.tile_pool(name="psO", bufs=cfg.get("psO", 2), space="PSUM"))
        psC = ctx.enter_context(tc.tile_pool(name="psC", bufs=cfg.get("psC", 1), space="PSUM"))

        # ---- constants into SBUF ----
        wcat_sb = consts.tile([P, 8, 49], dt.bfloat16)
        nc.sync.dma_start(wcat_sb[:], wcat_d.rearrange("(a p) m -> p a m", p=P))
        ballrow_sb = ones1_sb = None
        if not ball_trivial:
            ballrow_sb = consts.tile([1, 49], dt.bfloat16)
            nc.sync.dma_start(ballrow_sb[:], ballrow_d[:])
            ones1_sb = consts.tile([1, TC], dt.bfloat16)
            nc.vector.memset(ones1_sb[:], 1.0)
        reps_sb = consts.tile([48, 256], dt.bfloat16)
        nc.sync.dma_start(reps_sb[:], reps_d[:])
        repl_sb = consts.tile([48, 256], dt.bfloat16)
        nc.sync.dma_start(repl_sb[:], repl_d[:])
        tilel_sb = consts.tile([48, 256], dt.bfloat16)
        nc.sync.dma_start(tilel_sb[:], tilel_d[:])
        tilea_sb = consts.tile([48, 256], dt.bfloat16)
        nc.sync.dma_start(tilea_sb[:], tilea_d[:])
        rep16_sb = consts.tile([16, 256], dt.bfloat16)
        nc.sync.dma_start(rep16_sb[:], rep16_d[:])
        tile16_sb = consts.tile([16, 256], dt.bfloat16)
        nc.sync.dma_start(tile16_sb[:], tile16_d[:])
        g2_sb = consts.tile([128, 4, 16], dt.bfloat16)
        nc.sync.dma_start(g2_sb[:], g2c_d[:])
        a1p_sb = consts.tile([128, 128], dt.bfloat16)
        nc.sync.dma_start(a1p_sb[:], a1p_d[:])
        a2_sb = consts.tile([128, 128], dt.bfloat16)
        nc.sync.dma_start(a2_sb[:], a2_d[:])
        pcat_sb = consts.tile([66, DM], dt.bfloat16)
        nc.sync.dma_start(pcat_sb[:], pcat_d[:])
        svec_sb = consts.tile([48, 1], dt.bfloat16)
        nc.sync.dma_start(svec_sb[:], svec_d[:])
        sel49_sb = consts.tile([49, 1], dt.bfloat16)
        nc.sync.dma_start(sel49_sb[:], sel49_d[:])
        ident_sb = consts.tile([128, 128], dt.bfloat16)
        nc.sync.dma_start(ident_sb[:], ident_d[:])
        eps_sb = consts.tile([P, 1], dt.float32)
        nc.vector.memset(eps_sb[:], EPS)

        gb_sb = None
        if not (g_trivial and b_trivial):
            gb_sb = consts.tile([P, 2, DM], dt.float32)
            nc.sync.dma_start(gb_sb[:, 0, :], lng_d[None, :].to_broadcast((P, DM)))
            nc.sync.dma_start(gb_sb[:, 1, :], lnb_d[None, :].to_broadcast((P, DM)))

        rep_cm = tc.For_i(0, reps, 1) if reps > 1 else None
        if rep_cm is not None:
            rep_cm.__enter__()

        def expand_h(stat_sb, src_ap, nrows, h):
            ps_t = psE.tile([P, TC], dt.float32, tag="psE")
            nc.tensor.matmul(
                ps_t[:], stat_sb[0:nrows, bass.ts(h, 128)], src_ap
            )
            return ps_t

        def phase1(c):
            """Jv ladder for chunk c: load, transpose, project, associator."""
            t0 = c * TC
            csl = slice(t0, t0 + TC)

            x32 = xin.tile([P, TPC, DM], dt.float32)
            nc.sync.dma_start(
                x32[:], x_d[csl, :].rearrange("(j p) d -> p j d", p=P)
            )
            xb = xbp.tile([P, TPC, DM], dt.bfloat16)
            nc.gpsimd.tensor_copy(xb[:], x32[:])

            xT = xtp.tile([P, 8, TC], dt.bfloat16)
            for j in range(TPC):
                pxT = psX.tile([P, 8, 128], dt.bfloat16, tag="psX")
                for a in range(8):
                    nc.tensor.transpose(
                        pxT[:, a, :], xb[:, j, bass.ts(a, 128)], ident_sb[:]
                    )
                nc.scalar.copy(xT[:, :, bass.ts(j, 128)], pxT[:])

            pps = psP.tile([49, TC], dt.float32, tag="psP")
            for a in range(8):
                nc.tensor.matmul(
                    pps[:], wcat_sb[:, a, :], xT[:, a, :],
                    start=(a == 0), stop=(a == 7 and ball_trivial),
                )
            if not ball_trivial:
                nc.tensor.matmul(
                    pps[:], ballrow_sb[:], ones1_sb[:],
                    start=False, stop=True,
                )
            pall = mid.tile([49, TC], dt.bfloat16, tag="pall")
            nc.scalar.copy(pall[:], pps[:])
            p48 = pall[0:48, :]

            s_rep_ps = mid.tile([P, 2, TC], dt.bfloat16, tag="srp")
            for h in range(2):
                p = expand_h(reps_sb, p48, 48, h)
                nc.scalar.copy(s_rep_ps[:, h, :], p[:])
            s_tile_pa = mid.tile([P, 2, TC], dt.bfloat16, tag="stp")
            for h in range(2):
                p = expand_h(tilea_sb, p48, 48, h)
                nc.scalar.copy(s_tile_pa[:, h, :], p[:])

            w1 = mid.tile([P, 2, TC], dt.bfloat16, tag="w1")
            for h in range(2):
                p = expand_h(tilel_sb, p48, 48, h)
                nc.vector.tensor_mul(w1[:, h, :], s_rep_ps[:, h, :], p[:])
            w2 = mid.tile([P, 2, TC], dt.bfloat16, tag="w2")
            for h in range(2):
                p = expand_h(repl_sb, p48, 48, h)
                nc.vector.tensor_mul(w2[:, h, :], p[:], s_tile_pa[:, h, :])

            pU = psS.tile([16, TC], dt.float32, tag="psS")
            nc.tensor.matmul(pU[:], g2_sb[:, 0, :], w1[:, 0, :],
                             start=True, stop=False)
            nc.tensor.matmul(pU[:], g2_sb[:, 1, :], w1[:, 1, :],
                             start=False, stop=True)
            pY = psS.tile([16, TC], dt.float32, tag="psS")
            nc.tensor.matmul(pY[:], g2_sb[:, 0, :], w2[:, 0, :],
                             start=True, stop=False)
            nc.tensor.matmul(pY[:], g2_sb[:, 1, :], w2[:, 1, :],
                             start=False, stop=True)
            UY_sb = mid.tile([16, 2, TC], dt.bfloat16, tag="UY")
            nc.scalar.copy(UY_sb[:, 0, :], pU[:])
            nc.scalar.copy(UY_sb[:, 1, :], pY[:])

            w3 = mid.tile([P, 2, TC], dt.bfloat16, tag="w3")
            for h in range(2):
                p = expand_h(rep16_sb, UY_sb[:, 0, :], 16, h)
                nc.vector.tensor_mul(w3[:, h, :], p[:], s_tile_pa[:, h, :])
            w4 = mid.tile([P, 2, TC], dt.bfloat16, tag="w4")
            for h in range(2):
                p = expand_h(tile16_sb, UY_sb[:, 1, :], 16, h)
                nc.vector.tensor_mul(w4[:, h, :], s_rep_ps[:, h, :], p[:])

            pJv = psS.tile([16, TC], dt.float32, tag="psS")
            for i, (gi, w_t, h) in enumerate(
                [(0, w3, 0), (1, w3, 1), (2, w4, 0), (3, w4, 1)]
            ):
                nc.tensor.matmul(
                    pJv[:], g2_sb[:, gi, :], w_t[:, h, :],
                    start=(i == 0), stop=(i == 3),
                )

            TT = ttp.tile([66, TC], dt.bfloat16, tag="TT")
            nc.gpsimd.memset(TT[:], 0.0)
            nc.gpsimd.memset(TT[64:66, :], 1.0)
            nc.scalar.copy(TT[0:16, :], pJv[:])

            pjvT = psE.tile([P, TPC, 16], dt.bfloat16, tag="psE")
            for j in range(TPC):
                nc.tensor.transpose(
                    pjvT[:, j, :],
                    TT[0:16, bass.ts(j, 128)],
                    ident_sb[0:16, 0:16],
                )
            JvT = jvp.tile([P, TPC, 16], dt.bfloat16, tag="JvT")
            nc.vector.tensor_copy(JvT[:].bitcast(i32), pjvT[:].bitcast(i32))

            return dict(t0=t0, xb=xb, pall=pall, TT=TT, JvT=JvT)

        def phase2(st, st_prev):
            """Conv, final matmul, residual+LN, store for one chunk."""
            t0 = st["t0"]
            xb, pall, TT, JvT = st["xb"], st["pall"], st["TT"], st["JvT"]

            pJc = psC.tile([65, TC], dt.float32, tag="psC")
            for j in range(TPC):
                osl = pJc[32:48, bass.ts(j, 128)]
                if j > 0:
                    prev_stat = JvT[64:128, j - 1, :]
                elif st_prev is not None:
                    prev_stat = st_prev["JvT"][64:128, TPC - 1, :]
                else:
                    prev_stat = None
                if prev_stat is not None:
                    nc.tensor.matmul(
                        osl, prev_stat, a1p_sb[64:128, :],
                        start=True, stop=False, tile_position=(64, 32),
                    )
                    nc.tensor.matmul(
                        osl, JvT[:, j, :], a2_sb[:],
                        start=False, stop=True, tile_position=(0, 32),
                    )
                else:
                    nc.tensor.matmul(
                        osl, JvT[:, j, :], a2_sb[:],
                        start=True, stop=True, tile_position=(0, 32),
                    )
            nc.scalar.copy(TT[32:48, :], pJc[32:48, :])

            nc.tensor.matmul(
                pJc[64:65, :], svec_sb[:], TT[0:48, :],
                start=True, stop=False, tile_position=(0, 64),
            )
            nc.tensor.matmul(
                pJc[64:65, :], sel49_sb[:], pall[:],
                start=False, stop=True, tile_position=(0, 64),
            )
            nc.scalar.activation(
                TT[64:65, :], pJc[64:65, :],
                mybir.ActivationFunctionType.Copy,
                bias=-sumb / DM, scale=-1.0 / DM,
            )

            for j in range(TPC):
                ycb = ycp.tile([P, DM], dt.bfloat16, tag="ycb")
                for nh in range(2):
                    pout = psO.tile([P, 512], dt.float32, tag="psO")
                    nc.tensor.matmul(
                        pout[:],
                        TT[:, bass.ts(j, 128)],
                        pcat_sb[:, bass.ts(nh, 512)],
                    )
                    nc.vector.tensor_add(
                        ycb[:, bass.ts(nh, 512)],
                        xb[:, j, bass.ts(nh, 512)],
                        pout[:],
                    )
                ssq = stat.tile([P, 1], dt.float32, tag="ssq")
                sqs = stat.tile([P, DM], dt.bfloat16, tag="sqs")
                nc.scalar.activation(
                    sqs[:], ycb[:], mybir.ActivationFunctionType.Square,
                    accum_out=ssq[:],
                )
                sig = stat.tile([P, 1], dt.float32, tag="sig")
                nc.scalar.activation(
                    sig[:], ssq[:], mybir.ActivationFunctionType.Sqrt,
                    bias=eps_sb[:], scale=1.0 / DM,
                )
                rsig = stat.tile([P, 1], dt.float32, tag="rsig")
                nc.vector.reciprocal(rsig[:], sig[:])
                yout = yop.tile([P, DM], dt.float32, tag="yout")
                nc.vector.tensor_scalar_mul(yout[:], ycb[:], rsig[:])
                if gb_sb is not None:
                    nc.vector.tensor_mul(yout[:], yout[:], gb_sb[:, 0, :])
                    nc.vector.tensor_add(yout[:], yout[:], gb_sb[:, 1, :])
                nc.sync.dma_start(
                    y_d[t0 + j * P:t0 + (j + 1) * P, :], yout[:]
                )

        st_prev = None
        st_cur = None
        for c in range(NCH):
            st_new = phase1(c)
            if st_cur is not None:
                phase2(st_cur, st_prev)
            st_prev, st_cur = st_cur, st_new
        phase2(st_cur, st_prev)

        if rep_cm is not None:
            rep_cm.__exit__(None, None, None)

    return nc


# ----------------------------------------------------------------------------
# Entry point
# ----------------------------------------------------------------------------

def _const_map(fp):
    return {
        "wcat": fp["wcat"], "ballrow": fp["ballrow"],
        "rep_ps48": fp["rep_ps48"], "rep_pl48": fp["rep_pl48"],
        "tile_pl48": fp["tile_pl48"], "tile_pa48": fp["tile_pa48"],
        "rep16": fp["rep16"], "tile16": fp["tile16"],
        "g2c": fp["g2c"], "a1p": fp["a1p"], "a2": fp["a2"],
        "pcat": fp["pcat"], "svec": fp["svec"], "sel49": fp["sel49"],
        "ident": fp["ident"], "lng": fp["ln_g"], "lnb": fp["ln_b"],
    }


def _run(inputs, trace=False):
    x = inputs["x"]
    assert x.shape == (B, N, DM), x.shape
    fp = fold_params(inputs)

    nc = bacc.Bacc("TRN2", target_bir_lowering=False)
    build_kernel(nc, N, fp["sumb"], fp["g_trivial"], fp["b_trivial"],
                 ball_trivial=fp["ball_trivial"])
    nc.finalize()

    cm = _const_map(fp)
    in_maps = [
        {"x": np.ascontiguousarray(x[i], dtype=F32), **cm} for i in range(NCORES)
    ]
    return run_bass_kernel_spmd(nc, in_maps, list(range(NCORES)), trace=trace)


def kernel(**inputs):
    inputs = {k: np.asarray(v) for k, v in inputs.items()}
    res = _run(inputs)
    y = np.stack([res.results[i]["y"] for i in range(NCORES)], axis=0)
    return y.astype(np.float32)


def timed_run(inputs):
    """NTFF profiling is unavailable under axon in this container; timing is
    done by test.py via repeated execution of an in-kernel repeat loop."""
    return None


if __name__ == "__main__":
    import reference

    inp = reference.setup_inputs()
    out = kernel(**{k: np.asarray(v) for k, v in inp.items()})
    print("kernel output", out.shape, out.dtype)


# revision 28
# speedup vs baseline: 2.2498x; 1.1058x over previous
"""Trainium2 Bass kernel for nn_CausalFieldLayer (v2, software-pipelined).

Math (host-validated):
  * W_in folds into the three 1024->16 projections -> one [1024,49] matrix
    (48 proj cols + a ones column for sum_d x, used by the layernorm mean).
  * The complex-octonion associator Jv is computed per 512-token chunk in
    channel-major layout: PE expands ps/pl/pa (and U, Y) into 256-row
    outer-product operands via 48-row masked stationaries, DVE multiplies
    them, PE contracts by G2.
  * Everything downstream of Jv/Jc is linear and folds into pcat [66,1024];
    the 64-tap causal FFT conv is a Toeplitz matmul over token-major Jv.
  * Layernorm mean is folded into the final matmul as a -mu row; variance
    via ScalarE Square+accum; normalize on DVE. Data-parallel over B=8.

Performance structure (chosen via TimelineSim iteration):
  * Projections run as ONE 8-matmul group into contiguous pall [49,TC].
  * Residual path is bf16: x converted once on GPSIMD (otherwise-idle
    engine), feeding both the PE transposes and the residual add.
  * The chunk loop is software-pipelined: iteration c emits the Jv ladder
    (phase1) for chunk c and the conv/final/LN/store half (phase2) for
    chunk c-1, decoupling the two long dependency chains so the in-order
    engine queues always have independent work.
  * PSUM tags are phase-aligned (8 banks: psX 1 transposes, psE 2
    expansions+pjvT, psP 1 projections, psS 1 contractions, psO 2 final
    matmuls, psC 1 conv+mu) so a chunk's first PSUM alloc never waits on a
    late-prior-chunk release; pJc in particular must not share a tag with
    the ladder tiles or phase2(c) serializes behind phase1(c+1).
  * Per-j output tail (Square/sqrt/recip/normalize/DMA per 128-token tile)
    keeps the chain tail short.
"""
from contextlib import ExitStack

import numpy as np
import ml_dtypes

import concourse.bass as bass
import concourse.bacc as bacc
import concourse.mybir as mybir
import concourse.tile as tile
from concourse.bass_utils import run_bass_kernel_spmd

BF = ml_dtypes.bfloat16
F32 = np.float32

B, N, DM = 8, 4096, 1024
NCORES = 8
KSIZE = 64

EPS = 1e-5


# ----------------------------------------------------------------------------
# Host-side folding
# ----------------------------------------------------------------------------

def fold_params(inp):
    f64 = np.float64
    f = np.asarray(inp["oct_struct"], f64)  # [8,8,8] f[j,k,i]
    W_cat = np.concatenate(
        [np.asarray(inp[k], f64) for k in ("W_sigma", "W_lam", "W_alp")], axis=1
    )  # [1024,48]
    W_all = np.asarray(inp["W_in"], f64) @ W_cat
    b_all = np.asarray(inp["b_in"], f64) @ W_cat + np.concatenate(
        [np.asarray(inp[k], f64) for k in ("b_sigma", "b_lam", "b_alp")]
    )

    # cmul structure tensor G[i,j,k]: cmul(u,v)_i = sum_jk G[i,j,k] u_j v_k
    G = np.zeros((16, 16, 16), f64)
    ft = np.transpose(f, (2, 0, 1))  # ft[i,j,k] = f[j,k,i]
    G[:8, :8, :8] = ft
    G[:8, 8:, 8:] = -ft
    G[8:, :8, 8:] = ft
    G[8:, 8:, :8] = ft
    G2 = G.transpose(1, 2, 0).reshape(256, 16)  # [jk, i]

    JE = np.asarray(inp["J_expand"], f64)
    A = (JE - np.transpose(JE, (0, 2, 1))).reshape(16, 256)
    Gamma = np.einsum("ab,bcd->cd", np.asarray(inp["tetrad"], f64),
                      np.asarray(inp["gammas"], f64))
    sp = np.einsum("gdk,gd->k", np.asarray(inp["Pi_spinor"], f64), Gamma)
    PiS = np.asarray(inp["Pi_source"], f64).reshape(256, 16)
    PiT = np.asarray(inp["Pi_target"], f64).reshape(256, 16)
    C = (A @ PiS) @ PiT.T * np.tile(sp, 16)[None, :]

    kw = np.asarray(inp["kweights"], f64)
    alpha = kw[0]
    W_out = np.asarray(inp["W_out"], f64)
    P1 = alpha * (A @ W_out)
    P2 = (1.0 - alpha) * (C @ W_out)
    b_out = np.asarray(inp["b_out"], f64)

    # wcat [1024, 49]: ps@0:16, pl@16:32, pa@32:48, ones@48 (sum_d x)
    wcat = np.zeros((DM, 49), f64)
    wcat[:, 0:48] = W_all
    wcat[:, 48] = 1.0
    ball = np.zeros((49, 1), f64)
    ball[0:48, 0] = b_all

    # expansion stationaries reading contiguous pall[0:48]
    rep_ps48 = np.zeros((48, 256), f64)
    rep_pl48 = np.zeros((48, 256), f64)
    tile_pl48 = np.zeros((48, 256), f64)
    tile_pa48 = np.zeros((48, 256), f64)
    rep16 = np.zeros((16, 256), f64)
    tile16 = np.zeros((16, 256), f64)
    for j in range(16):
        for k in range(16):
            rep_ps48[j, j * 16 + k] = 1.0
            rep_pl48[16 + j, j * 16 + k] = 1.0
            tile_pl48[16 + k, j * 16 + k] = 1.0
            tile_pa48[32 + k, j * 16 + k] = 1.0
            rep16[j, j * 16 + k] = 1.0
            tile16[k, j * 16 + k] = 1.0

    # G2 chunks: [128, 4, 16] = [G2a, G2b, -G2a, -G2b]
    g2c = np.zeros((128, 4, 16), f64)
    g2c[:, 0] = G2[:128]
    g2c[:, 1] = G2[128:]
    g2c[:, 2] = -G2[:128]
    g2c[:, 3] = -G2[128:]

    # conv Toeplitz [192,128]: out[tl] = sum_sl afull[sl, tl] * Jv[t0-64+sl]
    afull = np.zeros((192, 128), f64)
    for sl in range(192):
        for tl in range(128):
            tap = tl + 64 - sl
            if 0 <= tap < KSIZE:
                afull[sl, tl] = kw[tap]
    a1p = np.zeros((128, 128), f64)
    a1p[64:128] = afull[0:64]  # stored at partition base 64
    a2 = afull[64:]

    # pcat [66, 1024]: 0:16 P1 (Jv), 32:48 P2 (Jc), 64 ones (-mu), 65 b_out
    pcat = np.zeros((66, DM), f64)
    pcat[0:16] = P1
    pcat[32:48] = P2
    pcat[64] = 1.0
    pcat[65] = b_out

    # svec [48,1]: row-sums of P1 at 0:16, of P2 at 32:48 (for sum_d out)
    svec = np.zeros((48, 1), f64)
    svec[0:16, 0] = P1.sum(axis=1)
    svec[32:48, 0] = P2.sum(axis=1)
    sumb = float(b_out.sum())

    # sel49 [49,1]: selects the sumx row (48) of pall
    sel49 = np.zeros((49, 1), f64)
    sel49[48, 0] = 1.0

    ln_g = np.asarray(inp["ln_g"], f64)
    ln_b = np.asarray(inp["ln_b"], f64)

    ballrow = ball.T  # [1, 49]

    return dict(
        ballrow=ballrow.astype(BF),
        ball_trivial=bool(np.all(ball == 0.0)),
        wcat=wcat.astype(BF),
        ball=ball.astype(F32),
        rep_ps48=rep_ps48.astype(BF),
        rep_pl48=rep_pl48.astype(BF),
        tile_pl48=tile_pl48.astype(BF),
        tile_pa48=tile_pa48.astype(BF),
        rep16=rep16.astype(BF),
        tile16=tile16.astype(BF),
        g2c=g2c.astype(BF),
        a1p=a1p.astype(BF),
        a2=a2.astype(BF),
        pcat=pcat.astype(BF),
        svec=svec.astype(BF),
        sel49=sel49.astype(BF),
        sumb=sumb,
        ident=np.eye(128).astype(BF),
        ln_g=ln_g.astype(F32),
        ln_b=ln_b.astype(F32),
        g_trivial=bool(np.all(ln_g == 1.0)),
        b_trivial=bool(np.all(ln_b == 0.0)),
    )


# ----------------------------------------------------------------------------
# Device kernel
# ----------------------------------------------------------------------------

def build_kernel(nc, T, sumb, g_trivial, b_trivial, reps=1,
                 ball_trivial=True, sb_bufs=2, cfg=None):
    cfg = cfg or {}
    dt = mybir.dt
    P = 128
    TC = 512                 # token chunk
    TPC = TC // P            # token tiles per chunk (4)
    NCH = T // TC            # chunks

    x_d = nc.declare_dram_parameter("x", [T, DM], dt.float32, isOutput=False)
    y_d = nc.declare_dram_parameter("y", [T, DM], dt.float32, isOutput=True)
    wcat_d = nc.declare_dram_parameter("wcat", [DM, 49], dt.bfloat16, isOutput=False)
    ballrow_d = nc.declare_dram_parameter("ballrow", [1, 49], dt.bfloat16, isOutput=False)
    reps_d = nc.declare_dram_parameter("rep_ps48", [48, 256], dt.bfloat16, isOutput=False)
    repl_d = nc.declare_dram_parameter("rep_pl48", [48, 256], dt.bfloat16, isOutput=False)
    tilel_d = nc.declare_dram_parameter("tile_pl48", [48, 256], dt.bfloat16, isOutput=False)
    tilea_d = nc.declare_dram_parameter("tile_pa48", [48, 256], dt.bfloat16, isOutput=False)
    rep16_d = nc.declare_dram_parameter("rep16", [16, 256], dt.bfloat16, isOutput=False)
    tile16_d = nc.declare_dram_parameter("tile16", [16, 256], dt.bfloat16, isOutput=False)
    g2c_d = nc.declare_dram_parameter("g2c", [128, 4, 16], dt.bfloat16, isOutput=False)
    a1p_d = nc.declare_dram_parameter("a1p", [128, 128], dt.bfloat16, isOutput=False)
    a2_d = nc.declare_dram_parameter("a2", [128, 128], dt.bfloat16, isOutput=False)
    pcat_d = nc.declare_dram_parameter("pcat", [66, DM], dt.bfloat16, isOutput=False)
    svec_d = nc.declare_dram_parameter("svec", [48, 1], dt.bfloat16, isOutput=False)
    sel49_d = nc.declare_dram_parameter("sel49", [49, 1], dt.bfloat16, isOutput=False)
    ident_d = nc.declare_dram_parameter("ident", [128, 128], dt.bfloat16, isOutput=False)
    lng_d = nc.declare_dram_parameter("lng", [DM], dt.float32, isOutput=False)
    lnb_d = nc.declare_dram_parameter("lnb", [DM], dt.float32, isOutput=False)

    i32 = dt.int32

    with tile.TileContext(nc) as tc, ExitStack() as ctx:
        consts = ctx.enter_context(tc.tile_pool(name="consts", bufs=1))
        xin = ctx.enter_context(tc.tile_pool(name="xin", bufs=cfg.get("xin", 2)))
        xbp = ctx.enter_context(tc.tile_pool(name="xbp", bufs=cfg.get("xbp", 3)))
        xtp = ctx.enter_context(tc.tile_pool(name="xtp", bufs=cfg.get("xtp", 2)))
        mid = ctx.enter_context(tc.tile_pool(name="mid", bufs=cfg.get("mid", 3)))
        ttp = ctx.enter_context(tc.tile_pool(name="ttp", bufs=cfg.get("ttp", 3)))
        jvp = ctx.enter_context(tc.tile_pool(name="jvp", bufs=cfg.get("jvp", 3)))
        prv = ctx.enter_context(tc.tile_pool(name="prv", bufs=cfg.get("prv", 2)))
        ycp = ctx.enter_context(tc.tile_pool(name="ycp", bufs=cfg.get("ycp", 4)))
        yop = ctx.enter_context(tc.tile_pool(name="yop", bufs=cfg.get("yop", 4)))
        stat = ctx.enter_context(tc.tile_pool(name="stat", bufs=cfg.get("stat", 4)))
        psX = ctx.enter_context(tc.tile_pool(name="psX", bufs=cfg.get("psX", 1), space="PSUM"))
        psE = ctx.enter_context(tc.tile_pool(name="psE", bufs=cfg.get("psE", 2), space="PSUM"))
        psP = ctx.enter_context(tc.tile_pool(name="psP", bufs=cfg.get("psP", 1), space="PSUM"))
        psS = ctx.enter_context(tc.tile_pool(name="psS", bufs=cfg.get("psS", 1), space="PSUM"))
        psO = ctx.enter_context(tc.tile_pool(name="psO", bufs=cfg.get("psO", 2), space="PSUM"))
        psC = ctx.enter_context(tc.tile_pool(name="psC", bufs=cfg.get("psC", 1), space="PSUM"))

        # ---- constants into SBUF ----
        wcat_sb = consts.tile([P, 8, 49], dt.bfloat16)
        nc.sync.dma_start(wcat_sb[:], wcat_d.rearrange("(a p) m -> p a m", p=P))
        ballrow_sb = ones1_sb = None
        if not ball_trivial:
            ballrow_sb = consts.tile([1, 49], dt.bfloat16)
            nc.sync.dma_start(ballrow_sb[:], ballrow_d[:])
            ones1_sb = consts.tile([1, TC], dt.bfloat16)
            nc.vector.memset(ones1_sb[:], 1.0)
        reps_sb = consts.tile([48, 256], dt.bfloat16)
        nc.sync.dma_start(reps_sb[:], reps_d[:])
        repl_sb = consts.tile([48, 256], dt.bfloat16)
        nc.sync.dma_start(repl_sb[:], repl_d[:])
        tilel_sb = consts.tile([48, 256], dt.bfloat16)
        nc.sync.dma_start(tilel_sb[:], tilel_d[:])
        tilea_sb = consts.tile([48, 256], dt.bfloat16)
        nc.sync.dma_start(tilea_sb[:], tilea_d[:])
        rep16_sb = consts.tile([16, 256], dt.bfloat16)
        nc.sync.dma_start(rep16_sb[:], rep16_d[:])
        tile16_sb = consts.tile([16, 256], dt.bfloat16)
        nc.sync.dma_start(tile16_sb[:], tile16_d[:])
        g2_sb = consts.tile([128, 4, 16], dt.bfloat16)
        nc.sync.dma_start(g2_sb[:], g2c_d[:])
        a1p_sb = consts.tile([128, 128], dt.bfloat16)
        nc.sync.dma_start(a1p_sb[:], a1p_d[:])
        a2_sb = consts.tile([128, 128], dt.bfloat16)
        nc.sync.dma_start(a2_sb[:], a2_d[:])
        pcat_sb = consts.tile([66, DM], dt.bfloat16)
        nc.sync.dma_start(pcat_sb[:], pcat_d[:])
        svec_sb = consts.tile([48, 1], dt.bfloat16)
        nc.sync.dma_start(svec_sb[:], svec_d[:])
        sel49_sb = consts.tile([49, 1], dt.bfloat16)
        nc.sync.dma_start(sel49_sb[:], sel49_d[:])
        ident_sb = consts.tile([128, 128], dt.bfloat16)
        nc.sync.dma_start(ident_sb[:], ident_d[:])
        eps_sb = consts.tile([P, 1], dt.float32)
        nc.vector.memset(eps_sb[:], EPS)

        gb_sb = None
        if not (g_trivial and b_trivial):
            gb_sb = consts.tile([P, 2, DM], dt.float32)
            nc.sync.dma_start(gb_sb[:, 0, :], lng_d[None, :].to_broadcast((P, DM)))
            nc.sync.dma_start(gb_sb[:, 1, :], lnb_d[None, :].to_broadcast((P, DM)))

        rep_cm = tc.For_i(0, reps, 1) if reps > 1 else None
        if rep_cm is not None:
            rep_cm.__enter__()

        def expand_h(stat_sb, src_ap, nrows, h):
            ps_t = psE.tile([P, TC], dt.float32, tag="psE")
            nc.tensor.matmul(
                ps_t[:], stat_sb[0:nrows, bass.ts(h, 128)], src_ap
            )
            return ps_t

        def phase1(c):
            """Jv ladder for chunk c: load, transpose, project, associator."""
            t0 = c * TC
            csl = slice(t0, t0 + TC)

            x32 = xin.tile([P, TPC, DM], dt.float32)
            nc.sync.dma_start(
                x32[:], x_d[csl, :].rearrange("(j p) d -> p j d", p=P)
            )
            xb = xbp.tile([P, TPC, DM], dt.bfloat16)
            nc.gpsimd.tensor_copy(xb[:], x32[:])

            xT = xtp.tile([P, 8, TC], dt.bfloat16)
            for j in range(TPC):
                pxT = psX.tile([P, 8, 128], dt.bfloat16, tag="psX")
                for a in range(8):
                    nc.tensor.transpose(
                        pxT[:, a, :], xb[:, j, bass.ts(a, 128)], ident_sb[:]
                    )
                nc.scalar.copy(xT[:, :, bass.ts(j, 128)], pxT[:])

            pps = psP.tile([49, TC], dt.float32, tag="psP")
            for a in range(8):
                nc.tensor.matmul(
                    pps[:], wcat_sb[:, a, :], xT[:, a, :],
                    start=(a == 0), stop=(a == 7 and ball_trivial),
                )
            if not ball_trivial:
                nc.tensor.matmul(
                    pps[:], ballrow_sb[:], ones1_sb[:],
                    start=False, stop=True,
                )
            pall = mid.tile([49, TC], dt.bfloat16, tag="pall")
            nc.scalar.copy(pall[:], pps[:])
            p48 = pall[0:48, :]

            s_rep_ps = mid.tile([P, 2, TC], dt.bfloat16, tag="srp")
            for h in range(2):
                p = expand_h(reps_sb, p48, 48, h)
                nc.scalar.copy(s_rep_ps[:, h, :], p[:])
            s_tile_pa = mid.tile([P, 2, TC], dt.bfloat16, tag="stp")
            for h in range(2):
                p = expand_h(tilea_sb, p48, 48, h)
                nc.scalar.copy(s_tile_pa[:, h, :], p[:])

            w1 = mid.tile([P, 2, TC], dt.bfloat16, tag="w1")
            for h in range(2):
                p = expand_h(tilel_sb, p48, 48, h)
                nc.vector.tensor_mul(w1[:, h, :], s_rep_ps[:, h, :], p[:])
            w2 = mid.tile([P, 2, TC], dt.bfloat16, tag="w2")
            for h in range(2):
                p = expand_h(repl_sb, p48, 48, h)
                nc.vector.tensor_mul(w2[:, h, :], p[:], s_tile_pa[:, h, :])

            pU = psS.tile([16, TC], dt.float32, tag="psS")
            nc.tensor.matmul(pU[:], g2_sb[:, 0, :], w1[:, 0, :],
                             start=True, stop=False)
            nc.tensor.matmul(pU[:], g2_sb[:, 1, :], w1[:, 1, :],
                             start=False, stop=True)
            pY = psS.tile([16, TC], dt.float32, tag="psS")
            nc.tensor.matmul(pY[:], g2_sb[:, 0, :], w2[:, 0, :],
                             start=True, stop=False)
            nc.tensor.matmul(pY[:], g2_sb[:, 1, :], w2[:, 1, :],
                             start=False, stop=True)
            UY_sb = mid.tile([16, 2, TC], dt.bfloat16, tag="UY")
            nc.scalar.copy(UY_sb[:, 0, :], pU[:])
            nc.scalar.copy(UY_sb[:, 1, :], pY[:])

            w3 = mid.tile([P, 2, TC], dt.bfloat16, tag="w3")
            for h in range(2):
                p = expand_h(rep16_sb, UY_sb[:, 0, :], 16, h)
                nc.vector.tensor_mul(w3[:, h, :], p[:], s_tile_pa[:, h, :])
            w4 = mid.tile([P, 2, TC], dt.bfloat16, tag="w4")
            for h in range(2):
                p = expand_h(tile16_sb, UY_sb[:, 1, :], 16, h)
                nc.vector.tensor_mul(w4[:, h, :], s_rep_ps[:, h, :], p[:])

            pJv = psS.tile([16, TC], dt.float32, tag="psS")
            for i, (gi, w_t, h) in enumerate(
                [(0, w3, 0), (1, w3, 1), (2, w4, 0), (3, w4, 1)]
            ):
                nc.tensor.matmul(
                    pJv[:], g2_sb[:, gi, :], w_t[:, h, :],
                    start=(i == 0), stop=(i == 3),
                )

            TT = ttp.tile([66, TC], dt.bfloat16, tag="TT")
            nc.gpsimd.memset(TT[:], 0.0)
            nc.gpsimd.memset(TT[64:66, :], 1.0)
            nc.scalar.copy(TT[0:16, :], pJv[:])

            pjvT = psE.tile([P, TPC, 16], dt.bfloat16, tag="psE")
            for j in range(TPC):
                nc.tensor.transpose(
                    pjvT[:, j, :],
                    TT[0:16, bass.ts(j, 128)],
                    ident_sb[0:16, 0:16],
                )
            JvT = jvp.tile([P, TPC, 16], dt.bfloat16, tag="JvT")
            nc.vector.tensor_copy(JvT[:].bitcast(i32), pjvT[:].bitcast(i32))

            return dict(t0=t0, xb=xb, pall=pall, TT=TT, JvT=JvT)

        def phase2(st, st_prev):
            """Conv, final matmul, residual+LN, store for one chunk."""
            t0 = st["t0"]
            xb, pall, TT, JvT = st["xb"], st["pall"], st["TT"], st["JvT"]

            pJc = psC.tile([65, TC], dt.float32, tag="psC")
            for j in range(TPC):
                osl = pJc[32:48, bass.ts(j, 128)]
                if j > 0:
                    prev_stat = JvT[64:128, j - 1, :]
                elif st_prev is not None:
                    prev_stat = st_prev["JvT"][64:128, TPC - 1, :]
                else:
                    prev_stat = None
                if prev_stat is not None:
                    nc.tensor.matmul(
                        osl, prev_stat, a1p_sb[64:128, :],
                        start=True, stop=False, tile_position=(64, 32),
                    )
                    nc.tensor.matmul(
                        osl, JvT[:, j, :], a2_sb[:],
                        start=False, stop=True, tile_position=(0, 32),
                    )
                else:
                    nc.tensor.matmul(
                        osl, JvT[:, j, :], a2_sb[:],
                        start=True, stop=True, tile_position=(0, 32),
                    )
            nc.scalar.copy(TT[32:48, :], pJc[32:48, :])

            nc.tensor.matmul(
                pJc[64:65, :], svec_sb[:], TT[0:48, :],
                start=True, stop=False, tile_position=(0, 64),
            )
            nc.tensor.matmul(
                pJc[64:65, :], sel49_sb[:], pall[:],
                start=False, stop=True, tile_position=(0, 64),
            )
            nc.scalar.activation(
                TT[64:65, :], pJc[64:65, :],
                mybir.ActivationFunctionType.Copy,
                bias=-sumb / DM, scale=-1.0 / DM,
            )

            for j in range(TPC):
                ycb = ycp.tile([P, DM], dt.bfloat16, tag="ycb")
                for nh in range(2):
                    pout = psO.tile([P, 512], dt.float32, tag="psO")
                    nc.tensor.matmul(
                        pout[:],
                        TT[:, bass.ts(j, 128)],
                        pcat_sb[:, bass.ts(nh, 512)],
                    )
                    nc.vector.tensor_add(
                        ycb[:, bass.ts(nh, 512)],
                        xb[:, j, bass.ts(nh, 512)],
                        pout[:],
                    )
                ssq = stat.tile([P, 1], dt.float32, tag="ssq")
                sqs = stat.tile([P, DM], dt.bfloat16, tag="sqs")
                nc.scalar.activation(
                    sqs[:], ycb[:], mybir.ActivationFunctionType.Square,
                    accum_out=ssq[:],
                )
                sig = stat.tile([P, 1], dt.float32, tag="sig")
                nc.scalar.activation(
                    sig[:], ssq[:], mybir.ActivationFunctionType.Sqrt,
                    bias=eps_sb[:], scale=1.0 / DM,
                )
                rsig = stat.tile([P, 1], dt.float32, tag="rsig")
                nc.vector.reciprocal(rsig[:], sig[:])
                yout = yop.tile([P, DM], dt.float32, tag="yout")
                nc.vector.tensor_scalar_mul(yout[:], ycb[:], rsig[:])
                if gb_sb is not None:
                    nc.vector.tensor_mul(yout[:], yout[:], gb_sb[:, 0, :])
                    nc.vector.tensor_add(yout[:], yout[:], gb_sb[:, 1, :])
                nc.sync.dma_start(
                    y_d[t0 + j * P:t0 + (j + 1) * P, :], yout[:]
                )

        st_prev = None
        st_cur = None
        for c in range(NCH):
            st_new = phase1(c)
            if st_cur is not None:
                phase2(st_cur, st_prev)
            st_prev, st_cur = st_cur, st_new
        phase2(st_cur, st_prev)

        if rep_cm is not None:
            rep_cm.__exit__(None, None, None)

    return nc


# ----------------------------------------------------------------------------
# Entry point
# ----------------------------------------------------------------------------

def _const_map(fp):
    return {
        "wcat": fp["wcat"], "ballrow": fp["ballrow"],
        "rep_ps48": fp["rep_ps48"], "rep_pl48": fp["rep_pl48"],
        "tile_pl48": fp["tile_pl48"], "tile_pa48": fp["tile_pa48"],
        "rep16": fp["rep16"], "tile16": fp["tile16"],
        "g2c": fp["g2c"], "a1p": fp["a1p"], "a2": fp["a2"],
        "pcat": fp["pcat"], "svec": fp["svec"], "sel49": fp["sel49"],
        "ident": fp["ident"], "lng": fp["ln_g"], "lnb": fp["ln_b"],
    }


def _run(inputs, trace=False):
    x = inputs["x"]
    assert x.shape == (B, N, DM), x.shape
    fp = fold_params(inputs)

    nc = bacc.Bacc("TRN2", target_bir_lowering=False)
    build_kernel(nc, N, fp["sumb"], fp["g_trivial"], fp["b_trivial"],
                 ball_trivial=fp["ball_trivial"])
    nc.finalize()

    cm = _const_map(fp)
    in_maps = [
        {"x": np.ascontiguousarray(x[i], dtype=F32), **cm} for i in range(NCORES)
    ]
    return run_bass_kernel_spmd(nc, in_maps, list(range(NCORES)), trace=trace)


def kernel(**inputs):
    inputs = {k: np.asarray(v) for k, v in inputs.items()}
    res = _run(inputs)
    y = np.stack([res.results[i]["y"] for i in range(NCORES)], axis=0)
    return y.astype(np.float32)


def timed_run(inputs):
    """NTFF profiling is unavailable under axon in this container; timing is
    done by test.py via repeated execution of an in-kernel repeat loop."""
    return None


if __name__ == "__main__":
    import reference

    inp = reference.setup_inputs()
    out = kernel(**{k: np.asarray(v) for k, v in inp.items()})
    print("kernel output", out.shape, out.dtype)
